# revision 6
# baseline (speedup 1.0000x reference)
"""Trainium2 Bass kernel for an 8-batch transformer encoder block.

Strategy: pure data parallelism -- batch B=8 across 8 NeuronCores, one
batch element (1024 tokens x 1024 dim) per core, full weights on every
core, no collectives.  All matmuls run in bf16 on the TensorEngine with
f32 PSUM accumulation; LayerNorm / softmax statistics stay f32.

Layout notes (per core):
  - Linear layers consume activations feature-major ("transposed",
    [C, tokens]) as the stationary matmul operand; transposes go through
    DRAM scratch using the DMA XBAR (bf16 2-byte transpose).
  - The reference reshapes (N, C) -> (H, N', hd) directly, so head h of
    q/k/v is the contiguous row-block [64h, 64h+64) of the projection
    reinterpreted as (1024, 64).  In the feature-major projection output
    qT[c, t] the per-head Q^T tile is gathered with 16 contiguous
    partition-shifted SBUF->SBUF copies; V comes from a natural-layout
    projection spilled to DRAM where each head is a contiguous (1024, 64)
    block.
  - Softmax is computed on S^T (keys on partitions) with no max
    subtraction (logits are ~N(0, 0.17), |logit| < ~3).  The softmax
    denominators come for free from a ones-column appended to V in the
    P@V matmul; normalization is a per-column scale applied to the
    64-row O^T head output.
"""

import sys

sys.path.insert(0, "/opt/trn_rl_repo")

import numpy as np
import ml_dtypes

import concourse.bass as bass
import concourse.tile as tile
from concourse import bacc, mybir

B, N, C, H = 8, 1024, 1024, 16
HD = C // H  # 64
HID = 4 * C  # 4096
P = 128
NT = N // P  # token chunks
CO = C // P  # feature chunks
JH = HID // P  # hidden chunks
EPS = 1e-5

F32 = mybir.dt.float32
BF16 = mybir.dt.bfloat16
AF = mybir.ActivationFunctionType
ALU = mybir.AluOpType

NCORES = 8

WEIGHT_NAMES = ["wq", "wk", "wv", "wp", "w1", "w2"]
VEC_NAMES = ["g1", "b1", "bq", "bk", "bv", "bp", "g2", "b2", "c1", "c2"]


def _ts(i, size):
    return slice(i * size, (i + 1) * size)


class _Pool:
    """Tile pool with manually controlled (non-LIFO) lifetime."""

    def __init__(self, tc, **kw):
        self._cm = tc.tile_pool(**kw)
        self.pool = self._cm.__enter__()

    _n = 0

    def tile(self, *a, **kw):
        if "name" not in kw:
            _Pool._n += 1
            kw["name"] = f"t{_Pool._n}"
        return self.pool.tile(*a, **kw)

    def close(self):
        self._cm.__exit__(None, None, None)


def build_program(nc):
    d = {}
    d["x"] = nc.dram_tensor("x", [N, C], F32, kind="ExternalInput").ap()
    for w, shape in [
        ("wq", [C, C]),
        ("wk", [C, C]),
        ("wv", [C, C]),
        ("wp", [C, C]),
        ("w1", [C, HID]),
        ("w2", [HID, C]),
    ]:
        d[w] = nc.dram_tensor(w, shape, BF16, kind="ExternalInput").ap()
    for v in VEC_NAMES:
        size = HID if v == "c1" else C
        d[v] = nc.dram_tensor(v, [size], F32, kind="ExternalInput").ap()
    d["out"] = nc.dram_tensor("out", [N, C], F32, kind="ExternalOutput").ap()

    h_d = nc.dram_tensor("h_nat", [N, C], BF16, kind="Internal").ap()
    v_d = nc.dram_tensor("v_nat", [N, C], BF16, kind="Internal").ap()
    h2_d = nc.dram_tensor("h2_nat", [N, C], BF16, kind="Internal").ap()

    with tile.TileContext(nc) as tc:
        _emit(tc, nc, d, h_d, v_d, h2_d)
    return nc


def _emit(tc, nc, d, h_d, v_d, h2_d):
    # ------- resident pools (level 0): tiny consts, psum, x1 -------
    consts = _Pool(tc, name="consts", bufs=1)
    bq_sb = consts.tile([P, CO], F32)
    nc.sync.dma_start(bq_sb[:], d["bq"].rearrange("(o p) -> p o", p=P))
    bk_sb = consts.tile([P, CO], F32)
    nc.sync.dma_start(bk_sb[:], d["bk"].rearrange("(o p) -> p o", p=P))
    c1_sb = consts.tile([P, JH], F32)
    nc.sync.dma_start(c1_sb[:], d["c1"].rearrange("(j p) -> p j", p=P))
    eps_sb = consts.tile([P, 1], F32)
    nc.vector.memset(eps_sb[:], EPS)

    # shared PSUM pool: all psum tiles are [128, 512] f32 (one bank each)
    psum = _Pool(tc, name="psum", bufs=6, space="PSUM")

    def ps_tile():
        return psum.tile([P, 512], F32, tag="mm", name="ps")

    x1_pool = _Pool(tc, name="x1", bufs=1)
    x1 = x1_pool.tile([P, NT, C], F32)

    def rep_tile(pool, vname):
        t = pool.tile([P, C], F32, tag=f"{vname}_rep", name=f"{vname}_rep")
        nc.sync.dma_start(t[:], d[vname].partition_broadcast(P))
        return t

    # ------- LayerNorm helper -------
    def layer_norm(work, src_ap, g_rep, b_rep, dst_dram, t):
        st = work.tile([P, 2, 6], F32, tag="ln_st", name="st")
        nc.vector.bn_stats(st[:, 0, :], src_ap[:, 0:512])
        nc.vector.bn_stats(st[:, 1, :], src_ap[:, 512:1024])
        mv = work.tile([P, 2], F32, tag="ln_mv", name="mv")
        nc.vector.bn_aggr(mv[:], st[:])
        rstd = work.tile([P, 1], F32, tag="ln_rstd", name="rstd")
        nc.scalar.activation(rstd[:], mv[:, 1:2], AF.Sqrt, bias=eps_sb[:, :])
        nc.vector.reciprocal(rstd[:], rstd[:])
        tmp = work.tile([P, C], F32, tag="ln_tmp", name="tmp")
        nc.vector.tensor_scalar(
            tmp[:], src_ap, mv[:, 0:1], rstd[:], op0=ALU.subtract, op1=ALU.mult
        )
        tmp2 = work.tile([P, C], F32, tag="ln_tmp2", name="tmp2")
        nc.vector.tensor_tensor(tmp2[:], tmp[:], g_rep[:], op=ALU.mult)
        hb = work.tile([P, C], BF16, tag="ln_out", name="hb")
        nc.vector.tensor_tensor(hb[:], tmp2[:], b_rep[:], op=ALU.add)
        nc.sync.dma_start(dst_dram[_ts(t, P), :], hb[:])

    # ------- level 1: wp, oT (live through proj); qkT (through attention) --
    wp_pool = _Pool(tc, name="wp", bufs=1)
    wp_sb = wp_pool.tile([P, CO, C], BF16)
    nc.sync.dma_start(wp_sb[:], d["wp"].rearrange("(o p) c -> p o c", p=P))
    oT_pool = _Pool(tc, name="oT", bufs=1)
    oT = oT_pool.tile([P, CO, N], BF16)
    qkT_pool = _Pool(tc, name="qkT", bufs=1)
    qT = qkT_pool.tile([P, CO, N], BF16)
    kT = qkT_pool.tile([P, CO, N], BF16)

    # ------- phase 1: LN1 -> h_nat (DRAM, bf16) -------
    ln1 = _Pool(tc, name="ln1", bufs=3)
    g1_rep = rep_tile(ln1, "g1")
    b1_rep = rep_tile(ln1, "b1")
    for t in range(NT):
        xt = ln1.tile([P, C], F32, tag="ln_x", name="xt")
        nc.sync.dma_start(xt[:], d["x"][_ts(t, P), :])
        layer_norm(ln1, xt[:], g1_rep, b1_rep, h_d, t)
    ln1.close()

    # ------- phase 2: hT via XBAR transpose -------
    hT_pool = _Pool(tc, name="hT", bufs=1)
    hT = hT_pool.tile([P, CO, N], BF16)
    for o in range(CO):
        nc.sync.dma_start_transpose(hT[:, o, :], h_d[:, _ts(o, P)])

    # ------- phase 3: QKV projections -------
    wqkv = _Pool(tc, name="wqkv", bufs=1)
    w_sb = {}
    for w in ["wq", "wk", "wv"]:
        w_sb[w] = wqkv.tile([P, CO, C], BF16, name=f"{w}_sb")
        nc.sync.dma_start(w_sb[w][:], d[w].rearrange("(o p) c -> p o c", p=P))

    # q/k: feature-major output qT[c_out, t], bias per partition via ACT
    for m in range(CO):
        for w, b_sb, dstT in (("wq", bq_sb, qT), ("wk", bk_sb, kT)):
            ps0 = ps_tile()
            ps1 = ps_tile()
            for o in range(CO):
                lhsT = w_sb[w][:, o, _ts(m, P)]
                nc.tensor.matmul(
                    ps0[:], lhsT, hT[:, o, 0:512], start=(o == 0), stop=(o == CO - 1)
                )
                nc.tensor.matmul(
                    ps1[:], lhsT, hT[:, o, 512:1024], start=(o == 0), stop=(o == CO - 1)
                )
            nc.scalar.activation(
                dstT[:, m, 0:512], ps0[:], AF.Identity, bias=b_sb[:, m : m + 1]
            )
            nc.scalar.activation(
                dstT[:, m, 512:1024], ps1[:], AF.Identity, bias=b_sb[:, m : m + 1]
            )

    # v: natural layout -> DRAM (per-head blocks become contiguous)
    vwork = _Pool(tc, name="vwork", bufs=2)
    bv_rep = rep_tile(vwork, "bv")
    for t in range(NT):
        ps0 = ps_tile()
        ps1 = ps_tile()
        for o in range(CO):
            lhsT = hT[:, o, _ts(t, P)]
            nc.tensor.matmul(
                ps0[:], lhsT, w_sb["wv"][:, o, 0:512], start=(o == 0), stop=(o == CO - 1)
            )
            nc.tensor.matmul(
                ps1[:],
                lhsT,
                w_sb["wv"][:, o, 512:1024],
                start=(o == 0),
                stop=(o == CO - 1),
            )
        vt = vwork.tile([P, C], BF16, tag="vt", name="vt")
        nc.vector.tensor_tensor(vt[:, 0:512], ps0[:], bv_rep[:, 0:512], op=ALU.add)
        nc.vector.tensor_tensor(
            vt[:, 512:1024], ps1[:], bv_rep[:, 512:1024], op=ALU.add
        )
        nc.sync.dma_start(v_d[_ts(t, P), :], vt[:])
    vwork.close()
    wqkv.close()
    hT_pool.close()

    # ------- phase 4: attention, head by head -------
    heads = _Pool(tc, name="heads", bufs=2)
    for h in range(H):
        # Q_h^T / K_h^T as [64 d, (16 beta, 64 alpha)]; attention position
        # n = 16*alpha + beta.  Source: qT[64*beta + dd, 64h + alpha].
        qh = heads.tile([HD, 16, HD], BF16, tag="qh", name="qh")
        kh = heads.tile([HD, 16, HD], BF16, tag="kh", name="kh")
        for b in range(16):
            sp = 64 * (b % 2)
            o = b // 2
            nc.sync.dma_start(qh[:, b, :], qT[sp : sp + HD, o, _ts(h, HD)])
            nc.sync.dma_start(kh[:, b, :], kT[sp : sp + HD, o, _ts(h, HD)])
        # V_h chunks [128 m, 64 d] + ones column for softmax denominators
        vh = heads.tile([P, 8, HD + 1], BF16, tag="vh", name="vh")
        nc.vector.memset(vh[:, :, HD : HD + 1], 1.0)
        for i in range(8):
            src = v_d[64 * h + 8 * i : 64 * h + 8 * i + 8, :].rearrange(
                "a (m dd) -> (a m) dd", dd=HD
            )
            nc.sync.dma_start(vh[:, i, 0:HD], src)

        # S^T = K_h Q_h^T (keys on partitions), exp via ACT (scale=1/8)
        est = heads.tile([P, 8, N], BF16, tag="est", name="est")
        for i in range(8):
            psa = ps_tile()
            psb = ps_tile()
            lhsT = kh[:, 2 * i : 2 * i + 2, :]  # [64, 128]
            nc.tensor.matmul(psa[:], lhsT, qh[:, 0:8, :], start=True, stop=True)
            nc.tensor.matmul(psb[:], lhsT, qh[:, 8:16, :], start=True, stop=True)
            nc.scalar.activation(est[:, i, 0:512], psa[:], AF.Exp, scale=0.125)
            nc.scalar.activation(est[:, i, 512:1024], psb[:], AF.Exp, scale=0.125)

        # O^T = [V|1]^T expS^T : rows 0..63 head output, row 64 denominators
        po0 = ps_tile()
        po1 = ps_tile()
        for i in range(8):
            nc.tensor.matmul(
                po0[0 : HD + 1, :],
                vh[:, i, :],
                est[:, i, 0:512],
                start=(i == 0),
                stop=(i == 7),
            )
            nc.tensor.matmul(
                po1[0 : HD + 1, :],
                vh[:, i, :],
                est[:, i, 512:1024],
                start=(i == 0),
                stop=(i == 7),
            )
        r = heads.tile([1, N], F32, tag="r", name="r")
        nc.vector.reciprocal(r[:, 0:512], po0[HD : HD + 1, :])
        nc.vector.reciprocal(r[:, 512:1024], po1[HD : HD + 1, :])
        rr = heads.tile([HD, N], F32, tag="rr", name="rr")
        nc.gpsimd.partition_broadcast(rr[:], r[:], channels=HD)

        # normalize + un-permute (beta, alpha) -> n = 16*alpha + beta
        p0 = HD * (h % 2)
        oc = h // 2
        for half, po in ((0, po0), (1, po1)):
            dst = oT[p0 : p0 + HD, oc, :].rearrange("p (a b2) -> p b2 a", b2=16)[
                :, 8 * half : 8 * half + 8, :
            ]
            src_ps = po[0:HD, :].rearrange("p (b2 a) -> p b2 a", b2=8)
            src_rr = rr[:, _ts(half, 512)].rearrange("p (b2 a) -> p b2 a", b2=8)
            nc.vector.tensor_tensor(dst, src_ps, src_rr, op=ALU.mult)
    heads.close()
    qkT_pool.close()

    # ------- phase 5: proj + residual -> x1 ; LN2 -> h2_nat -------
    ln2 = _Pool(tc, name="ln2", bufs=3)
    g2_rep = rep_tile(ln2, "g2")
    b2_rep = rep_tile(ln2, "b2")
    bp_rep = rep_tile(ln2, "bp")
    for t in range(NT):
        ps0 = ps_tile()
        ps1 = ps_tile()
        for o in range(CO):
            lhsT = oT[:, o, _ts(t, P)]
            nc.tensor.matmul(
                ps0[:], lhsT, wp_sb[:, o, 0:512], start=(o == 0), stop=(o == CO - 1)
            )
            nc.tensor.matmul(
                ps1[:], lhsT, wp_sb[:, o, 512:1024], start=(o == 0), stop=(o == CO - 1)
            )
        xt = ln2.tile([P, C], F32, tag="ln_x", name="xt")
        nc.sync.dma_start(xt[:], d["x"][_ts(t, P), :])
        nc.vector.tensor_tensor(
            x1[:, t, 0:512], ps0[:], bp_rep[:, 0:512], op=ALU.add
        )
        nc.vector.tensor_tensor(
            x1[:, t, 512:1024], ps1[:], bp_rep[:, 512:1024], op=ALU.add
        )
        nc.vector.tensor_tensor(x1[:, t, :], x1[:, t, :], xt[:], op=ALU.add)
        layer_norm(ln2, x1[:, t, :], g2_rep, b2_rep, h2_d, t)
    ln2.close()
    oT_pool.close()
    wp_pool.close()

    # ------- phase 6/7: h2T via XBAR; FC1 + exact GELU -> m1T -------
    m1_pool = _Pool(tc, name="m1T", bufs=1)
    m1T = m1_pool.tile([P, JH, N], BF16)
    h2T_pool = _Pool(tc, name="h2T", bufs=1)
    h2T = h2T_pool.tile([P, CO, N], BF16)
    for o in range(CO):
        nc.sync.dma_start_transpose(h2T[:, o, :], h2_d[:, _ts(o, P)])

    w1s = _Pool(tc, name="w1s", bufs=3)
    w1_r = d["w1"].rearrange("(o p) c -> p o c", p=P)
    for j in range(JH):
        w1t = w1s.tile([P, CO, P], BF16, tag="w1t", name="w1t")
        nc.sync.dma_start(w1t[:], w1_r[:, :, _ts(j, P)])
        ps0 = ps_tile()
        ps1 = ps_tile()
        for o in range(CO):
            nc.tensor.matmul(
                ps0[:], w1t[:, o, :], h2T[:, o, 0:512], start=(o == 0), stop=(o == CO - 1)
            )
            nc.tensor.matmul(
                ps1[:],
                w1t[:, o, :],
                h2T[:, o, 512:1024],
                start=(o == 0),
                stop=(o == CO - 1),
            )
        nc.scalar.activation(
            m1T[:, j, 0:512], ps0[:], AF.Gelu, bias=c1_sb[:, j : j + 1]
        )
        nc.scalar.activation(
            m1T[:, j, 512:1024], ps1[:], AF.Gelu, bias=c1_sb[:, j : j + 1]
        )
    w1s.close()
    h2T_pool.close()

    # ------- phase 8: FC2 (4 hid blocks) + residual -> out -------
    acc_pool = _Pool(tc, name="acc", bufs=1)
    acc = acc_pool.tile([P, NT, C], F32)
    w2s = _Pool(tc, name="w2s", bufs=1)
    ow = _Pool(tc, name="ow", bufs=2)
    c2_rep = rep_tile(ow, "c2")
    w2_r = d["w2"].rearrange("(j p) c -> p j c", p=P)
    NBLK = 4
    JB = JH // NBLK  # 8
    for blk in range(NBLK):
        w2b = w2s.tile([P, JB, C], BF16, tag="w2b", name="w2b")
        nc.sync.dma_start(w2b[:], w2_r[:, _ts(blk, JB), :])
        for t in range(NT):
            ps0 = ps_tile()
            ps1 = ps_tile()
            for jj in range(JB):
                j = blk * JB + jj
                lhsT = m1T[:, j, _ts(t, P)]
                nc.tensor.matmul(
                    ps0[:], lhsT, w2b[:, jj, 0:512], start=(jj == 0), stop=(jj == JB - 1)
                )
                nc.tensor.matmul(
                    ps1[:],
                    lhsT,
                    w2b[:, jj, 512:1024],
                    start=(jj == 0),
                    stop=(jj == JB - 1),
                )
            if blk == 0:
                nc.vector.tensor_copy(acc[:, t, 0:512], ps0[:])
                nc.vector.tensor_copy(acc[:, t, 512:1024], ps1[:])
            elif blk < NBLK - 1:
                nc.vector.tensor_tensor(
                    acc[:, t, 0:512], acc[:, t, 0:512], ps0[:], op=ALU.add
                )
                nc.vector.tensor_tensor(
                    acc[:, t, 512:1024], acc[:, t, 512:1024], ps1[:], op=ALU.add
                )
            else:
                ot = ow.tile([P, C], F32, tag="ot", name="ot")
                nc.vector.tensor_tensor(
                    ot[:, 0:512], acc[:, t, 0:512], ps0[:], op=ALU.add
                )
                nc.vector.tensor_tensor(
                    ot[:, 512:1024], acc[:, t, 512:1024], ps1[:], op=ALU.add
                )
                nc.vector.tensor_tensor(ot[:], ot[:], c2_rep[:], op=ALU.add)
                nc.vector.tensor_tensor(ot[:], ot[:], x1[:, t, :], op=ALU.add)
                nc.sync.dma_start(d["out"][_ts(t, P), :], ot[:])
    ow.close()
    w2s.close()
    acc_pool.close()
    m1_pool.close()
    x1_pool.close()
    psum.close()
    consts.close()


_CACHE = {}


def get_nc():
    if "nc" not in _CACHE:
        nc = bacc.Bacc(
            "TRN2", target_bir_lowering=False, debug=False, num_devices=NCORES
        )
        build_program(nc)
        nc.compile()
        _CACHE["nc"] = nc
    return _CACHE["nc"]


def make_in_maps(inputs):
    bf = lambda a: np.ascontiguousarray(np.asarray(a, np.float32)).astype(
        ml_dtypes.bfloat16
    )
    f32 = lambda a: np.ascontiguousarray(np.asarray(a, np.float32))
    shared = {
        "wq": bf(inputs["Wq"]),
        "wk": bf(inputs["Wk"]),
        "wv": bf(inputs["Wv"]),
        "wp": bf(inputs["Wp"]),
        "w1": bf(inputs["W1"]),
        "w2": bf(inputs["W2"]),
        "g1": f32(inputs["g1"]),
        "b1": f32(inputs["b1"]),
        "bq": f32(inputs["bq"]),
        "bk": f32(inputs["bk"]),
        "bv": f32(inputs["bv"]),
        "bp": f32(inputs["bp"]),
        "g2": f32(inputs["g2"]),
        "b2": f32(inputs["b2"]),
        "c1": f32(inputs["c1"]),
        "c2": f32(inputs["c2"]),
    }
    x = np.asarray(inputs["x"], np.float32)
    return [{**shared, "x": np.ascontiguousarray(x[c])} for c in range(NCORES)]


def kernel(**inputs):
    from concourse.bass_utils import run_bass_kernel_spmd

    nc = get_nc()
    in_maps = make_in_maps(inputs)
    res = run_bass_kernel_spmd(nc, in_maps, core_ids=list(range(NCORES)))
    out = np.stack(
        [np.asarray(res.results[c]["out"], np.float32) for c in range(NCORES)], axis=0
    )
    return out


# revision 16
# speedup vs baseline: 1.0349x; 1.0349x over previous
"""Trainium2 Bass kernel for an 8-batch transformer encoder block.

Strategy: pure data parallelism -- batch B=8 across 8 NeuronCores, one
batch element (1024 tokens x 1024 dim) per core, full weights on every
core, no collectives.  All matmuls run in bf16 on the TensorEngine with
f32 PSUM accumulation; LayerNorm / softmax statistics stay f32.

Layout notes (per core):
  - Linear layers consume activations feature-major ("transposed",
    [C, tokens]) as the stationary matmul operand; transposes go through
    DRAM scratch using the DMA XBAR (bf16 2-byte transpose).
  - The reference reshapes (N, C) -> (H, N', hd) directly, so head h of
    q/k/v is the contiguous row-block [64h, 64h+64) of the projection
    reinterpreted as (1024, 64).  In the feature-major projection output
    qT[c, t] the per-head Q^T tile is gathered with 16 contiguous
    partition-shifted SBUF->SBUF copies; V comes from a natural-layout
    projection spilled to DRAM where each head is a contiguous (1024, 64)
    block.
  - Softmax is computed on S^T (keys on partitions) with no max
    subtraction (logits are ~N(0, 0.17), |logit| < ~3).  The softmax
    denominators come for free from a ones-column appended to V in the
    P@V matmul; normalization is a per-column scale applied to the
    64-row O^T head output.
"""

import os
import sys

sys.path.insert(0, "/opt/trn_rl_repo")

import numpy as np
import ml_dtypes

import concourse.bass as bass
import concourse.tile as tile
from concourse import bacc, mybir

B, N, C, H = 8, 1024, 1024, 16
HD = C // H  # 64
HID = 4 * C  # 4096
P = 128
NT = N // P  # token chunks
CO = C // P  # feature chunks
JH = HID // P  # hidden chunks
EPS = 1e-5

F32 = mybir.dt.float32
BF16 = mybir.dt.bfloat16
AF = mybir.ActivationFunctionType
ALU = mybir.AluOpType

NCORES = 8

WEIGHT_NAMES = ["wq", "wk", "wv", "wp", "w1", "w2"]
VEC_NAMES = ["g1", "b1", "bq", "bk", "bv", "bp", "g2", "b2", "c1", "c2"]


def _ts(i, size):
    return slice(i * size, (i + 1) * size)


class _Pool:
    """Tile pool with manually controlled (non-LIFO) lifetime."""

    def __init__(self, tc, **kw):
        self._cm = tc.tile_pool(**kw)
        self.pool = self._cm.__enter__()

    _n = 0

    def tile(self, *a, **kw):
        if "name" not in kw:
            _Pool._n += 1
            kw["name"] = f"t{_Pool._n}"
        return self.pool.tile(*a, **kw)

    def close(self):
        self._cm.__exit__(None, None, None)


def build_program(nc):
    d = {}
    d["x"] = nc.dram_tensor("x", [N, C], F32, kind="ExternalInput").ap()
    for w, shape in [
        ("wq", [C, C]),
        ("wk", [C, C]),
        ("wv", [C, C]),
        ("wp", [C, C]),
        ("w1", [C, HID]),
        ("w2", [HID, C]),
    ]:
        d[w] = nc.dram_tensor(w, shape, BF16, kind="ExternalInput").ap()
    for v in VEC_NAMES:
        size = HID if v == "c1" else C
        d[v] = nc.dram_tensor(v, [size], F32, kind="ExternalInput").ap()
    d["out"] = nc.dram_tensor("out", [N, C], F32, kind="ExternalOutput").ap()

    h_d = nc.dram_tensor("h_nat", [N, C], BF16, kind="Internal").ap()
    v_d = nc.dram_tensor("v_nat", [N, C], BF16, kind="Internal").ap()
    h2_d = nc.dram_tensor("h2_nat", [N, C], BF16, kind="Internal").ap()

    debug = bool(os.environ.get("KERNEL_DEBUG_TAPS"))
    dbg = {}
    if debug:
        for nm, shape, dt in [
            ("dbg_h", [N, C], BF16),
            ("dbg_v", [N, C], BF16),
            ("dbg_h2", [N, C], BF16),
            ("dbg_qT", [P, CO, N], BF16),
            ("dbg_kT", [P, CO, N], BF16),
            ("dbg_oT", [P, CO, N], BF16),
            ("dbg_x1", [P, NT, C], F32),
        ]:
            dbg[nm] = nc.dram_tensor(nm, shape, dt, kind="ExternalOutput").ap()

    with tile.TileContext(nc) as tc:
        _emit(tc, nc, d, h_d, v_d, h2_d, dbg)
    return nc


def _emit(tc, nc, d, h_d, v_d, h2_d, dbg=None):
    dbg = dbg or {}
    # ------- resident pools (level 0): tiny consts, psum, x1 -------
    consts = _Pool(tc, name="consts", bufs=1)
    bq_sb = consts.tile([P, CO], F32)
    nc.sync.dma_start(bq_sb[:], d["bq"].rearrange("(o p) -> p o", p=P))
    bk_sb = consts.tile([P, CO], F32)
    nc.sync.dma_start(bk_sb[:], d["bk"].rearrange("(o p) -> p o", p=P))
    c1_sb = consts.tile([P, JH], F32)
    nc.sync.dma_start(c1_sb[:], d["c1"].rearrange("(j p) -> p j", p=P))
    eps_sb = consts.tile([P, 1], F32)
    nc.vector.memset(eps_sb[:], EPS)

    # shared PSUM pool: all psum tiles are [128, 512] f32 (one bank each)
    psum = _Pool(tc, name="psum", bufs=6, space="PSUM")

    def ps_tile():
        return psum.tile([P, 512], F32, tag="mm", name="ps")

    x1_pool = _Pool(tc, name="x1", bufs=1)
    x1 = x1_pool.tile([P, NT, C], F32)

    def rep_tile(pool, vname):
        t = pool.tile([P, C], F32, tag=f"{vname}_rep", name=f"{vname}_rep")
        nc.sync.dma_start(t[:], d[vname].partition_broadcast(P))
        return t

    # ------- LayerNorm helper -------
    def layer_norm(work, src_ap, g_rep, b_rep, dst_dram, t):
        st = work.tile([P, 2, 6], F32, tag="ln_st", name="st")
        nc.vector.bn_stats(st[:, 0, :], src_ap[:, 0:512])
        nc.vector.bn_stats(st[:, 1, :], src_ap[:, 512:1024])
        mv = work.tile([P, 2], F32, tag="ln_mv", name="mv")
        nc.vector.bn_aggr(mv[:], st[:])
        rstd = work.tile([P, 1], F32, tag="ln_rstd", name="rstd")
        nc.scalar.activation(rstd[:], mv[:, 1:2], AF.Sqrt, bias=eps_sb[:, :])
        nc.vector.reciprocal(rstd[:], rstd[:])
        tmp = work.tile([P, C], F32, tag="ln_tmp", name="tmp")
        nc.vector.tensor_scalar(
            tmp[:], src_ap, mv[:, 0:1], rstd[:], op0=ALU.subtract, op1=ALU.mult
        )
        tmp2 = work.tile([P, C], F32, tag="ln_tmp2", name="tmp2")
        nc.vector.tensor_tensor(tmp2[:], tmp[:], g_rep[:], op=ALU.mult)
        hb = work.tile([P, C], BF16, tag="ln_out", name="hb")
        nc.vector.tensor_tensor(hb[:], tmp2[:], b_rep[:], op=ALU.add)
        nc.sync.dma_start(dst_dram[_ts(t, P), :], hb[:])

    # ------- level 1: wp, oT (live through proj); qkT (through attention) --
    wp_pool = _Pool(tc, name="wp", bufs=1)
    wp_sb = wp_pool.tile([P, CO, C], BF16)
    nc.sync.dma_start(wp_sb[:], d["wp"].rearrange("(o p) c -> p o c", p=P))
    oT_pool = _Pool(tc, name="oT", bufs=1)
    oT = oT_pool.tile([P, CO, N], BF16)
    qkT_pool = _Pool(tc, name="qkT", bufs=1)
    qT = qkT_pool.tile([P, CO, N], BF16)
    kT = qkT_pool.tile([P, CO, N], BF16)

    # ------- phase 1: LN1 -> h_nat (DRAM, bf16) -------
    ln1 = _Pool(tc, name="ln1", bufs=3)
    g1_rep = rep_tile(ln1, "g1")
    b1_rep = rep_tile(ln1, "b1")
    for t in range(NT):
        xt = ln1.tile([P, C], F32, tag="ln_x", name="xt")
        nc.sync.dma_start(xt[:], d["x"][_ts(t, P), :])
        layer_norm(ln1, xt[:], g1_rep, b1_rep, h_d, t)
    ln1.close()

    # ------- phase 2: hT via XBAR transpose -------
    hT_pool = _Pool(tc, name="hT", bufs=1)
    hT = hT_pool.tile([P, CO, N], BF16)
    for o in range(CO):
        nc.sync.dma_start_transpose(hT[:, o, :], h_d[:, _ts(o, P)])

    # ------- phase 3: QKV projections -------
    wqkv = _Pool(tc, name="wqkv", bufs=1)
    w_sb = {}
    for w in ["wq", "wk", "wv"]:
        w_sb[w] = wqkv.tile([P, CO, C], BF16, name=f"{w}_sb")
        nc.sync.dma_start(w_sb[w][:], d[w].rearrange("(o p) c -> p o c", p=P))

    # q/k: feature-major output qT[c_out, t], bias per partition via ACT
    for m in range(CO):
        for w, b_sb, dstT in (("wq", bq_sb, qT), ("wk", bk_sb, kT)):
            ps0 = ps_tile()
            ps1 = ps_tile()
            for o in range(CO):
                lhsT = w_sb[w][:, o, _ts(m, P)]
                nc.tensor.matmul(
                    ps0[:], lhsT, hT[:, o, 0:512], start=(o == 0), stop=(o == CO - 1)
                )
                nc.tensor.matmul(
                    ps1[:], lhsT, hT[:, o, 512:1024], start=(o == 0), stop=(o == CO - 1)
                )
            nc.scalar.activation(
                dstT[:, m, 0:512], ps0[:], AF.Identity, bias=b_sb[:, m : m + 1]
            )
            nc.scalar.activation(
                dstT[:, m, 512:1024], ps1[:], AF.Identity, bias=b_sb[:, m : m + 1]
            )

    # v: natural layout -> DRAM (per-head blocks become contiguous)
    vwork = _Pool(tc, name="vwork", bufs=2)
    bv_rep = rep_tile(vwork, "bv")
    for t in range(NT):
        ps0 = ps_tile()
        ps1 = ps_tile()
        for o in range(CO):
            lhsT = hT[:, o, _ts(t, P)]
            nc.tensor.matmul(
                ps0[:], lhsT, w_sb["wv"][:, o, 0:512], start=(o == 0), stop=(o == CO - 1)
            )
            nc.tensor.matmul(
                ps1[:],
                lhsT,
                w_sb["wv"][:, o, 512:1024],
                start=(o == 0),
                stop=(o == CO - 1),
            )
        vt = vwork.tile([P, C], BF16, tag="vt", name="vt")
        nc.vector.tensor_tensor(vt[:, 0:512], ps0[:], bv_rep[:, 0:512], op=ALU.add)
        nc.vector.tensor_tensor(
            vt[:, 512:1024], ps1[:], bv_rep[:, 512:1024], op=ALU.add
        )
        nc.sync.dma_start(v_d[_ts(t, P), :], vt[:])
    vwork.close()
    wqkv.close()
    hT_pool.close()

    if dbg:
        nc.sync.dma_start(dbg["dbg_h"], h_d)
        nc.sync.dma_start(dbg["dbg_v"], v_d)
        nc.sync.dma_start(dbg["dbg_qT"], qT[:])
        nc.sync.dma_start(dbg["dbg_kT"], kT[:])

    # ------- phase 4: attention, head by head -------
    heads = _Pool(tc, name="heads", bufs=2)
    for h in range(H):
        # Q_h^T / K_h^T as [64 d, (16 beta, 64 alpha)]; attention position
        # n = 16*alpha + beta.  Source: qT[64*beta + dd, 64h + alpha].
        qh = heads.tile([HD, 16, HD], BF16, tag="qh", name="qh")
        kh = heads.tile([HD, 16, HD], BF16, tag="kh", name="kh")
        for b in range(16):
            sp = 64 * (b % 2)
            o = b // 2
            nc.sync.dma_start(qh[:, b, :], qT[sp : sp + HD, o, _ts(h, HD)])
            nc.sync.dma_start(kh[:, b, :], kT[sp : sp + HD, o, _ts(h, HD)])
        # V_h chunks + ones column for softmax denominators.  Chunk i holds
        # m-values with m%16 in {2i, 2i+1} at partition p = 64*bb + a'
        # (m = 16a' + 2i + bb), matching the S^T psum partition order below.
        vh = heads.tile([P, 8, HD + 1], BF16, tag="vh", name="vh")
        nc.vector.memset(vh[:, :, HD : HD + 1], 1.0)
        vblk = v_d[64 * h : 64 * h + 64, :].rearrange("t (g dd) -> g t dd", dd=HD)
        for i in range(8):
            for bb in range(2):
                nc.sync.dma_start(
                    vh[64 * bb : 64 * bb + 64, i, 0:HD], vblk[2 * i + bb]
                )

        # S^T = K_h Q_h^T (keys on partitions), exp via ACT (scale=1/8).
        # psum partition p = 64*(b'%2) + a' <-> m = 16a' + 2i + b'%2.
        est = heads.tile([P, 8, N], BF16, tag="est", name="est")
        for i in range(8):
            psa = ps_tile()
            psb = ps_tile()
            lhsT = kh[:, 2 * i : 2 * i + 2, :]  # [64, 128]
            nc.tensor.matmul(psa[:], lhsT, qh[:, 0:8, :], start=True, stop=True)
            nc.tensor.matmul(psb[:], lhsT, qh[:, 8:16, :], start=True, stop=True)
            nc.scalar.activation(est[:, i, 0:512], psa[:], AF.Exp, scale=0.125)
            nc.scalar.activation(est[:, i, 512:1024], psb[:], AF.Exp, scale=0.125)

        # O^T = [V|1]^T expS^T : rows 0..63 head output, row 64 denominators
        po0 = ps_tile()
        po1 = ps_tile()
        for i in range(8):
            nc.tensor.matmul(
                po0[0 : HD + 1, :],
                vh[:, i, :],
                est[:, i, 0:512],
                start=(i == 0),
                stop=(i == 7),
            )
            nc.tensor.matmul(
                po1[0 : HD + 1, :],
                vh[:, i, :],
                est[:, i, 512:1024],
                start=(i == 0),
                stop=(i == 7),
            )
        r = heads.tile([1, N], F32, tag="r", name="r")
        nc.vector.reciprocal(r[:, 0:512], po0[HD : HD + 1, :])
        nc.vector.reciprocal(r[:, 512:1024], po1[HD : HD + 1, :])
        rr = heads.tile([HD, N], F32, tag="rr", name="rr")
        nc.gpsimd.partition_broadcast(rr[:], r[:], channels=HD)

        # normalize + un-permute (beta, alpha) -> n = 16*alpha + beta
        p0 = HD * (h % 2)
        oc = h // 2
        for half, po in ((0, po0), (1, po1)):
            dst = oT[p0 : p0 + HD, oc, :].rearrange("p (a b2) -> p b2 a", b2=16)[
                :, 8 * half : 8 * half + 8, :
            ]
            src_ps = po[0:HD, :].rearrange("p (b2 a) -> p b2 a", b2=8)
            src_rr = rr[:, _ts(half, 512)].rearrange("p (b2 a) -> p b2 a", b2=8)
            nc.vector.tensor_tensor(dst, src_ps, src_rr, op=ALU.mult)
    heads.close()
    qkT_pool.close()
    if dbg:
        nc.sync.dma_start(dbg["dbg_oT"], oT[:])

    # ------- phase 5: proj + residual -> x1 ; LN2 -> h2_nat -------
    ln2 = _Pool(tc, name="ln2", bufs=3)
    g2_rep = rep_tile(ln2, "g2")
    b2_rep = rep_tile(ln2, "b2")
    bp_rep = rep_tile(ln2, "bp")
    for t in range(NT):
        ps0 = ps_tile()
        ps1 = ps_tile()
        for o in range(CO):
            lhsT = oT[:, o, _ts(t, P)]
            nc.tensor.matmul(
                ps0[:], lhsT, wp_sb[:, o, 0:512], start=(o == 0), stop=(o == CO - 1)
            )
            nc.tensor.matmul(
                ps1[:], lhsT, wp_sb[:, o, 512:1024], start=(o == 0), stop=(o == CO - 1)
            )
        xt = ln2.tile([P, C], F32, tag="ln_x", name="xt")
        nc.sync.dma_start(xt[:], d["x"][_ts(t, P), :])
        nc.vector.tensor_tensor(
            x1[:, t, 0:512], ps0[:], bp_rep[:, 0:512], op=ALU.add
        )
        nc.vector.tensor_tensor(
            x1[:, t, 512:1024], ps1[:], bp_rep[:, 512:1024], op=ALU.add
        )
        nc.vector.tensor_tensor(x1[:, t, :], x1[:, t, :], xt[:], op=ALU.add)
        layer_norm(ln2, x1[:, t, :], g2_rep, b2_rep, h2_d, t)
    ln2.close()
    oT_pool.close()
    wp_pool.close()
    if dbg:
        nc.sync.dma_start(dbg["dbg_x1"], x1[:])
        nc.sync.dma_start(dbg["dbg_h2"], h2_d)

    # ------- phase 6/7: h2T via XBAR; FC1 + exact GELU -> m1T -------
    m1_pool = _Pool(tc, name="m1T", bufs=1)
    m1T = m1_pool.tile([P, JH, N], BF16)
    h2T_pool = _Pool(tc, name="h2T", bufs=1)
    h2T = h2T_pool.tile([P, CO, N], BF16)
    for o in range(CO):
        nc.sync.dma_start_transpose(h2T[:, o, :], h2_d[:, _ts(o, P)])

    w1s = _Pool(tc, name="w1s", bufs=3)
    w1_r = d["w1"].rearrange("(o p) c -> p o c", p=P)
    for j in range(JH):
        w1t = w1s.tile([P, CO, P], BF16, tag="w1t", name="w1t")
        nc.sync.dma_start(w1t[:], w1_r[:, :, _ts(j, P)])
        ps0 = ps_tile()
        ps1 = ps_tile()
        for o in range(CO):
            nc.tensor.matmul(
                ps0[:], w1t[:, o, :], h2T[:, o, 0:512], start=(o == 0), stop=(o == CO - 1)
            )
            nc.tensor.matmul(
                ps1[:],
                w1t[:, o, :],
                h2T[:, o, 512:1024],
                start=(o == 0),
                stop=(o == CO - 1),
            )
        nc.scalar.activation(
            m1T[:, j, 0:512], ps0[:], AF.Gelu, bias=c1_sb[:, j : j + 1]
        )
        nc.scalar.activation(
            m1T[:, j, 512:1024], ps1[:], AF.Gelu, bias=c1_sb[:, j : j + 1]
        )
    w1s.close()
    h2T_pool.close()

    # ------- phase 8: FC2 (4 hid blocks) + residual -> out -------
    acc_pool = _Pool(tc, name="acc", bufs=1)
    acc = acc_pool.tile([P, NT, C], F32)
    w2s = _Pool(tc, name="w2s", bufs=1)
    ow = _Pool(tc, name="ow", bufs=2)
    c2_rep = rep_tile(ow, "c2")
    w2_r = d["w2"].rearrange("(j p) c -> p j c", p=P)
    NBLK = 4
    JB = JH // NBLK  # 8
    for blk in range(NBLK):
        w2b = w2s.tile([P, JB, C], BF16, tag="w2b", name="w2b")
        nc.sync.dma_start(w2b[:], w2_r[:, _ts(blk, JB), :])
        for t in range(NT):
            ps0 = ps_tile()
            ps1 = ps_tile()
            for jj in range(JB):
                j = blk * JB + jj
                lhsT = m1T[:, j, _ts(t, P)]
                nc.tensor.matmul(
                    ps0[:], lhsT, w2b[:, jj, 0:512], start=(jj == 0), stop=(jj == JB - 1)
                )
                nc.tensor.matmul(
                    ps1[:],
                    lhsT,
                    w2b[:, jj, 512:1024],
                    start=(jj == 0),
                    stop=(jj == JB - 1),
                )
            if blk == 0:
                nc.vector.tensor_copy(acc[:, t, 0:512], ps0[:])
                nc.vector.tensor_copy(acc[:, t, 512:1024], ps1[:])
            elif blk < NBLK - 1:
                nc.vector.tensor_tensor(
                    acc[:, t, 0:512], acc[:, t, 0:512], ps0[:], op=ALU.add
                )
                nc.vector.tensor_tensor(
                    acc[:, t, 512:1024], acc[:, t, 512:1024], ps1[:], op=ALU.add
                )
            else:
                ot = ow.tile([P, C], F32, tag="ot", name="ot")
                nc.vector.tensor_tensor(
                    ot[:, 0:512], acc[:, t, 0:512], ps0[:], op=ALU.add
                )
                nc.vector.tensor_tensor(
                    ot[:, 512:1024], acc[:, t, 512:1024], ps1[:], op=ALU.add
                )
                nc.vector.tensor_tensor(ot[:], ot[:], c2_rep[:], op=ALU.add)
                nc.vector.tensor_tensor(ot[:], ot[:], x1[:, t, :], op=ALU.add)
                nc.sync.dma_start(d["out"][_ts(t, P), :], ot[:])
    ow.close()
    w2s.close()
    acc_pool.close()
    m1_pool.close()
    x1_pool.close()
    psum.close()
    consts.close()


_CACHE = {}


def get_nc():
    if "nc" not in _CACHE:
        nc = bacc.Bacc(
            "TRN2", target_bir_lowering=False, debug=False, num_devices=NCORES
        )
        build_program(nc)
        nc.compile()
        _CACHE["nc"] = nc
    return _CACHE["nc"]


def make_in_maps(inputs):
    bf = lambda a: np.ascontiguousarray(np.asarray(a, np.float32)).astype(
        ml_dtypes.bfloat16
    )
    f32 = lambda a: np.ascontiguousarray(np.asarray(a, np.float32))
    shared = {
        "wq": bf(inputs["Wq"]),
        "wk": bf(inputs["Wk"]),
        "wv": bf(inputs["Wv"]),
        "wp": bf(inputs["Wp"]),
        "w1": bf(inputs["W1"]),
        "w2": bf(inputs["W2"]),
        "g1": f32(inputs["g1"]),
        "b1": f32(inputs["b1"]),
        "bq": f32(inputs["bq"]),
        "bk": f32(inputs["bk"]),
        "bv": f32(inputs["bv"]),
        "bp": f32(inputs["bp"]),
        "g2": f32(inputs["g2"]),
        "b2": f32(inputs["b2"]),
        "c1": f32(inputs["c1"]),
        "c2": f32(inputs["c2"]),
    }
    x = np.asarray(inputs["x"], np.float32)
    return [{**shared, "x": np.ascontiguousarray(x[c])} for c in range(NCORES)]


def kernel(**inputs):
    from concourse.bass_utils import run_bass_kernel_spmd

    nc = get_nc()
    in_maps = make_in_maps(inputs)
    res = run_bass_kernel_spmd(nc, in_maps, core_ids=list(range(NCORES)))
    out = np.stack(
        [np.asarray(res.results[c]["out"], np.float32) for c in range(NCORES)], axis=0
    )
    return out
